# revision 8
# baseline (speedup 1.0000x reference)
"""Correlation cost-volume kernel for Trainium2 (Bass/Tile).

Problem: in1, in2: [B=8, C=128, H=96, W=128] fp32.
Output: [B, 81, H, W] where out[b, dy*9+dx, y, x] =
    mean_c( in1[b,c,y,x] * in2_pad[b,c,y+dy,x+dx] ),
with in2 zero-padded by 4 in both spatial dims (max_displacement=4).

Strategy (data-parallel over batch, one sample per NeuronCore):
  - Host pre-scales in1 by 1/C (exact in fp16: 128 = 2^7) and converts
    both inputs to fp16 -> PE streams at 1 col/cycle (4x the fp32 rate)
    and input HBM traffic is halved.
  - For each in1 row y, compute the Gram band against the 9 surrounding
    (padded) in2 rows with 3 TensorE matmuls: stationary = in1[:, y, :]
    ([C=128, W=128]), moving = in2p rows y+3j..y+3j+2 ([C, 3x136]) ->
    PSUM G[x, j, (r, x')] = sum_c in1[c,y,x]/C * in2p[c, y+3j+r, x'].
  - Copy PSUM->SBUF in 32-partition groups, keeping only the 40-wide
    window Wt[x, dy, u] = G[x, dy, 32*(x//32)+u] each pixel group needs
    (pure access patterns only: mixed partition+byte strides in DMA APs
    miscompute on HW - the DGE wraps the per-partition byte carry),
    casting to fp16 on the way (PSUM is fp32).
  - Ship the windowed tiles to DRAM with one large contiguous DMA per
    R-row batch. The banded gather t2[x,y,dy*9+dx] = Wt[x,y,dy,s+dx]
    (s = x mod 32) is a pure layout permutation with zero FLOPs; doing
    it on-device costs ~110K 36-byte DMA descriptors, so it is done on
    the host with a numpy stride-tricks view during unsharding.
"""

import numpy as np

import concourse.bass as bass
import concourse.mybir as mybir
from concourse import bacc
from concourse.bass_utils import run_bass_kernel_spmd
from concourse.tile import TileContext

B = 8
C = 128
H = 96
W = 128
D = 9  # 2*max_disp + 1
K = D * D  # 81 output channels
PAD = 4
WP = W + 2 * PAD  # 136
FP32 = mybir.dt.float32
FP16 = mybir.dt.float16

N_CORES = 8
R = 16  # rows per store batch
WIN = 72  # per-64-partition-group window width (64 + 8)


def build_bass(h: int = H):
    """Build the per-core Bass program for a [C, h, W] sample."""
    hp = h + 2 * PAD
    nbatch = (h + R - 1) // R
    assert h % R == 0
    nc = bacc.Bacc(None, target_bir_lowering=False)
    in1 = nc.dram_tensor("in1", [C, h, W], FP16, kind="ExternalInput")
    # in2p is host-padded: [C, h+8, W+8] with zeros in the 4-wide borders.
    in2p = nc.dram_tensor("in2p", [C, hp, WP], FP16, kind="ExternalInput")
    # windowed Gram bands; host extracts the diagonal band + transposes.
    wtd = nc.dram_tensor("wtd", [W, h, D, WIN], FP16, kind="ExternalOutput")

    with TileContext(nc) as tc:
        with (
            tc.tile_pool(name="big", bufs=1) as big_pool,
            tc.tile_pool(name="wtp", bufs=3) as wt_pool,
            tc.tile_pool(name="gpsum", bufs=2, space="PSUM") as gpsum,
        ):
            s1 = big_pool.tile([C, h, W], FP16, name="s1")
            s2p = big_pool.tile([C, hp, WP], FP16, name="s2p")

            # Load inputs in row-chunks so compute can start early.
            for i in range(0, h, R):
                nc.sync.dma_start(s1[:, i : i + R, :], in1[:, i : i + R, :])
            rows2 = R + 2
            for i in range(0, hp, rows2):
                r = min(rows2, hp - i)
                nc.gpsimd.dma_start(s2p[:, i : i + r, :], in2p[:, i : i + r, :])

            for b in range(nbatch):
                wt = wt_pool.tile([128, R, D, WIN], FP16, name="wt", tag="wt")

                for r in range(R):
                    y = b * R + r
                    # --- 6 matmuls: per (group g, dy-triplet j) ---
                    # moving = in2p rows y+3j..y+3j+2, cols [64g, 64g+72)
                    # (216 cols). Group g's three windows pack into banks
                    # 2g..2g+1 (slots of 256 fp32), so the two groups'
                    # PSUM->SBUF copies touch disjoint banks and run in
                    # parallel on scalar/vector, and each copy only waits
                    # for its own group's matmuls.
                    gp6 = gpsum.tile([128, 2, 4, 256], FP32, name="gp6", tag="gp")
                    for g in range(2):
                        for j in range(3):
                            nc.tensor.matmul(
                                gp6[:, g, j, 0 : 3 * WIN],
                                s1[:, y, :],
                                s2p[
                                    :,
                                    y + 3 * j : y + 3 * j + 3,
                                    64 * g : 64 * g + WIN,
                                ],
                                start=True,
                                stop=True,
                            )

                    # --- PSUM -> SBUF windowed copy (per 64-part group) ---
                    # Wt[x, dy, u] = G[x, dy, 64*(x//64) + u], u in [0, 72)
                    wt_r = wt[:, r, :, :].rearrange("p (j r2) u -> p j r2 u", j=3)
                    for g in range(2):
                        src = gp6[
                            64 * g : 64 * g + 64, g, 0:3, 0 : 3 * WIN
                        ].rearrange("p j (r2 u) -> p j r2 u", u=WIN)
                        dst = wt_r[64 * g : 64 * g + 64, :, :, :]
                        if g == 0:
                            nc.scalar.activation(
                                dst, src, mybir.ActivationFunctionType.Copy
                            )
                        else:
                            nc.vector.tensor_copy(dst, src)

                # --- store the batch: contiguous per-partition runs ---
                eng = (nc.sync, nc.gpsimd)[b % 2]
                eng.dma_start(wtd[:, b * R : (b + 1) * R, :, :], wt[:, :, :, :])

    nc.compile()
    return nc


_cached = {}


def _get_nc(h: int):
    if h not in _cached:
        _cached[h] = build_bass(h)
    return _cached[h]


def _prep_in1(in1: np.ndarray) -> np.ndarray:
    # fold the mean's 1/C into in1; 1/128 is a power of two so the
    # fp16 rounding of in1 itself is unaffected.
    return np.ascontiguousarray((in1 * (1.0 / C)).astype(np.float16))


def _pad_in2(in2: np.ndarray) -> np.ndarray:
    # [C, h, W] -> [C, h+8, W+8] zero-padded, contiguous fp16
    return np.ascontiguousarray(
        np.pad(
            in2.astype(np.float16), ((0, 0), (PAD, PAD), (PAD, PAD)),
            mode="constant",
        )
    )


def _extract(wtd: np.ndarray) -> np.ndarray:
    """[128, h, 9, WIN] windowed bands -> [81, h, 128] cost volume (fp32).

    out[dy*9+dx, y, 64g+s] = wtd[64g+s, y, dy, s+dx]
    """
    w = np.ascontiguousarray(wtd)
    h = w.shape[1]
    sx, sy, sdy, su = w.strides
    v = np.lib.stride_tricks.as_strided(
        w,
        shape=(2, 64, h, D, D),
        strides=(64 * sx, sx + su, sy, sdy, su),
    )
    # v[g, s, y, dy, dx] = w[64g+s, y, dy, s+dx]
    return (
        v.transpose(3, 4, 2, 0, 1).reshape(K, h, W).astype(np.float32)
    )


def kernel(**inputs: np.ndarray) -> np.ndarray:
    in1 = np.ascontiguousarray(inputs["in1"], dtype=np.float32)
    in2 = np.ascontiguousarray(inputs["in2"], dtype=np.float32)
    assert in1.shape == (B, C, H, W), in1.shape

    nc = _get_nc(H)
    in_maps = [
        {
            "in1": _prep_in1(in1[b]),
            "in2p": _pad_in2(in2[b]),
        }
        for b in range(B)
    ]
    res = run_bass_kernel_spmd(nc, in_maps, core_ids=list(range(N_CORES)))
    return np.stack([_extract(r["wtd"]) for r in res.results], axis=0)


# revision 10
# speedup vs baseline: 1.7325x; 1.7325x over previous
"""Correlation cost-volume kernel for Trainium2 (Bass/Tile).

Problem: in1, in2: [B=8, C=128, H=96, W=128] fp32.
Output: [B, 81, H, W] where out[b, dy*9+dx, y, x] =
    mean_c( in1[b,c,y,x] * in2_pad[b,c,y+dy,x+dx] ),
with in2 zero-padded by 4 in both spatial dims (max_displacement=4).

Strategy (data-parallel over batch, one sample per NeuronCore):
  - Host pre-scales in1 by 1/C (exact in fp16: 128 = 2^7) and converts
    both inputs to fp16 -> PE streams at 1 col/cycle (4x the fp32 rate)
    and input HBM traffic is halved.
  - For each in1 row y, compute the Gram band against the 9 surrounding
    (padded) in2 rows with 3 TensorE matmuls: stationary = in1[:, y, :]
    ([C=128, W=128]), moving = in2p rows y+3j..y+3j+2 ([C, 3x136]) ->
    PSUM G[x, j, (r, x')] = sum_c in1[c,y,x]/C * in2p[c, y+3j+r, x'].
  - Copy PSUM->SBUF in 32-partition groups, keeping only the 40-wide
    window Wt[x, dy, u] = G[x, dy, 32*(x//32)+u] each pixel group needs
    (pure access patterns only: mixed partition+byte strides in DMA APs
    miscompute on HW - the DGE wraps the per-partition byte carry),
    casting to fp16 on the way (PSUM is fp32).
  - Ship the windowed tiles to DRAM with one large contiguous DMA per
    R-row batch. The banded gather t2[x,y,dy*9+dx] = Wt[x,y,dy,s+dx]
    (s = x mod 32) is a pure layout permutation with zero FLOPs; doing
    it on-device costs ~110K 36-byte DMA descriptors, so it is done on
    the host with a numpy stride-tricks view during unsharding.
"""

import numpy as np

import concourse.bass as bass
import concourse.mybir as mybir
from concourse import bacc
from concourse.bass_utils import run_bass_kernel_spmd
from concourse.tile import TileContext

B = 8
C = 128
H = 96
W = 128
D = 9  # 2*max_disp + 1
K = D * D  # 81 output channels
PAD = 4
WP = W + 2 * PAD  # 136
FP32 = mybir.dt.float32
FP16 = mybir.dt.float16

N_CORES = 8
R = 16  # rows per store batch
WIN = 72  # per-64-partition-group window width (64 + 8)


def build_bass(h: int = H):
    """Build the per-core Bass program for a [C, h, W] sample."""
    hp = h + 2 * PAD
    nbatch = (h + R - 1) // R
    assert h % R == 0
    nc = bacc.Bacc(None, target_bir_lowering=False)
    in1 = nc.dram_tensor("in1", [C, h, W], FP16, kind="ExternalInput")
    # in2p is host-padded: [C, h+8, W+8] with zeros in the 4-wide borders.
    in2p = nc.dram_tensor("in2p", [C, hp, WP], FP16, kind="ExternalInput")
    # windowed Gram bands; host extracts the diagonal band + transposes.
    wtd = nc.dram_tensor("wtd", [W, h, D, WIN], FP16, kind="ExternalOutput")

    with TileContext(nc) as tc:
        with (
            tc.tile_pool(name="big", bufs=1) as big_pool,
            tc.tile_pool(name="wtp", bufs=3) as wt_pool,
            tc.tile_pool(name="gps0", bufs=2, space="PSUM") as gps0,
            tc.tile_pool(name="gps1", bufs=2, space="PSUM") as gps1,
        ):
            s1 = big_pool.tile([C, h, W], FP16, name="s1")
            s2p = big_pool.tile([C, hp, WP], FP16, name="s2p")

            # Load inputs in row-chunks so compute can start early.
            for i in range(0, h, R):
                nc.sync.dma_start(s1[:, i : i + R, :], in1[:, i : i + R, :])
            rows2 = R + 2
            for i in range(0, hp, rows2):
                r = min(rows2, hp - i)
                nc.gpsimd.dma_start(s2p[:, i : i + r, :], in2p[:, i : i + r, :])

            for b in range(nbatch):
                wt = wt_pool.tile([128, R, D, WIN], FP16, name="wt", tag="wt")

                for r in range(R):
                    y = b * R + r
                    # --- 6 matmuls: per (group g, dy-triplet j) ---
                    # moving = in2p rows y+3j..y+3j+2, cols [64g, 64g+72)
                    # (216 cols). Each group's three windows pack into its
                    # own 2-bank PSUM tile (slots of 256 fp32), so the two
                    # groups' PSUM->SBUF copies touch disjoint banks and
                    # run in parallel on scalar/vector, and each copy only
                    # waits for its own group's matmuls. Emission order:
                    # g1 MMs -> DVE copy (slower engine first), g0 MMs ->
                    # ACT copy.
                    wt_r = wt[:, r, :, :].rearrange("p (j r2) u -> p j r2 u", j=3)
                    for g in (1, 0):
                        pool = gps1 if g == 1 else gps0
                        gpg = pool.tile(
                            [128, 3, 256], FP32, name=f"gp{g}", tag=f"gp{g}"
                        )
                        for j in range(3):
                            nc.tensor.matmul(
                                gpg[:, j, 0 : 3 * WIN],
                                s1[:, y, :],
                                s2p[
                                    :,
                                    y + 3 * j : y + 3 * j + 3,
                                    64 * g : 64 * g + WIN,
                                ],
                                start=True,
                                stop=True,
                            )
                        # --- PSUM -> SBUF windowed copy (64-part group) ---
                        # Wt[x, dy, u] = G[x, dy, 64*(x//64)+u], u in [0,72)
                        src = gpg[
                            64 * g : 64 * g + 64, 0:3, 0 : 3 * WIN
                        ].rearrange("p j (r2 u) -> p j r2 u", u=WIN)
                        dst = wt_r[64 * g : 64 * g + 64, :, :, :]
                        if g == 0:
                            nc.scalar.activation(
                                dst, src, mybir.ActivationFunctionType.Copy
                            )
                        else:
                            nc.vector.tensor_copy(dst, src)

                # --- store the batch: contiguous per-partition runs ---
                eng = (nc.sync, nc.gpsimd)[b % 2]
                eng.dma_start(wtd[:, b * R : (b + 1) * R, :, :], wt[:, :, :, :])

    nc.compile()
    return nc


_cached = {}


def _get_nc(h: int):
    if h not in _cached:
        _cached[h] = build_bass(h)
    return _cached[h]


def _prep_in1(in1: np.ndarray) -> np.ndarray:
    # fold the mean's 1/C into in1; 1/128 is a power of two so the
    # fp16 rounding of in1 itself is unaffected.
    return np.ascontiguousarray((in1 * (1.0 / C)).astype(np.float16))


def _pad_in2(in2: np.ndarray) -> np.ndarray:
    # [C, h, W] -> [C, h+8, W+8] zero-padded, contiguous fp16
    return np.ascontiguousarray(
        np.pad(
            in2.astype(np.float16), ((0, 0), (PAD, PAD), (PAD, PAD)),
            mode="constant",
        )
    )


def _extract(wtd: np.ndarray) -> np.ndarray:
    """[128, h, 9, WIN] windowed bands -> [81, h, 128] cost volume (fp32).

    out[dy*9+dx, y, 64g+s] = wtd[64g+s, y, dy, s+dx]
    """
    w = np.ascontiguousarray(wtd)
    h = w.shape[1]
    sx, sy, sdy, su = w.strides
    v = np.lib.stride_tricks.as_strided(
        w,
        shape=(2, 64, h, D, D),
        strides=(64 * sx, sx + su, sy, sdy, su),
    )
    # v[g, s, y, dy, dx] = w[64g+s, y, dy, s+dx]
    return (
        v.transpose(3, 4, 2, 0, 1).reshape(K, h, W).astype(np.float32)
    )


def kernel(**inputs: np.ndarray) -> np.ndarray:
    in1 = np.ascontiguousarray(inputs["in1"], dtype=np.float32)
    in2 = np.ascontiguousarray(inputs["in2"], dtype=np.float32)
    assert in1.shape == (B, C, H, W), in1.shape

    nc = _get_nc(H)
    in_maps = [
        {
            "in1": _prep_in1(in1[b]),
            "in2p": _pad_in2(in2[b]),
        }
        for b in range(B)
    ]
    res = run_bass_kernel_spmd(nc, in_maps, core_ids=list(range(N_CORES)))
    return np.stack([_extract(r["wtd"]) for r in res.results], axis=0)
